# revision 17
# baseline (speedup 1.0000x reference)
"""Gumbel Top-K gate kernel for Trainium2 (8 NeuronCores, SPMD).

Math: mask[b, 0, r, m] = 1 iff z[b, r, m] is among the top-16 of row r, where
  z = mean_h(q_h k_h^T)/sqrt(64) + gumbel(u),  gumbel = -log(-log(u+eps)+eps).
Softmax is strictly monotone per row, so the reference's softmax/top-k mask
equals thresholding z at its 16th-largest value per row (ties included via >=).

Engine split (the DVE and GpSimd share an SBUF port, so GpSimd offload is
counterproductive — GpSimd is unused; ACT has its own ports):
  ACT: g1 = Ln(u+eps); g2 = Ln(-g1+eps); final compare as
       mask_u8 = Sign(z - t16m) (Sign clamps {-1,0,1} -> {0,1} on the u8
       output path; t16m is 1-2 ulp below t16 so z == t16 lands at 1).
  PE:  16 fp16 matmuls per 128-row tile (q pre-scaled by 1/8 on host; PSUM
       holds 8*logits).
  DVE: one fused scalar_tensor_tensor z = (PSUM*0.125) - g2 (PSUM
       evacuation + head-mean + gumbel combine in one 1x pass), then the
       top-k search: 8x max8 over 256-col segments -> 64 candidates, then
       max8/match_replace/max8 on the candidates -> t16 = 16th largest.

Segmented max8 is exact unless one segment holds >8 of the row's top-16
(P ~ 4e-5 per row). Measured flips vs the fp32 reference on the actual
inputs: 7 of 16.7M (rel err 7.3e-3, budget 2e-2) — from the fp16 matmul
(6) and the segment assumption (1).

The mask stage for tile t is emitted after tile t+1's Ln passes (manual
software pipelining) so the in-order ACT queue never stalls waiting for the
DVE chain. Activation tables are pinned to one set to avoid reload thrash.

Sharding: core c handles batch b = c//2, row half c%2 -> [1024, 2048] slab.
"""

import sys

sys.path.insert(0, "/opt/trn_rl_repo")

import numpy as np

import concourse.bacc as bacc
import concourse.mybir as mybir
import concourse.tile as tile
from concourse import bass_utils

B, H, N, D = 4, 8, 2048, 64
HD = H * D  # 512 contraction dim (heads concatenated)
N_CORES = 8
ROWS = N * B // N_CORES  # 1024 rows per core
P = 128
EPS = 1e-9
NEG_BIG = -3.0e38
NSEG = 8
SEG = N // NSEG  # 256
# t16m = t16 * (1 - 2^-22): 1-2 ulp below t16 (t16 > 0 w.o.p.), so
# Sign(z - t16m) is +1 exactly when z >= t16.
T16_SHRINK = -(1.0 - 2.0**-22)  # negated: used directly as the Sign bias
F32 = mybir.dt.float32
F16 = mybir.dt.float16
U8 = mybir.dt.uint8


def _pin_act_tables(arch):
    """Pin Ln (and Exp, if ever used) to the combined table set so the ACT
    engine never reloads tables mid-kernel; Sign is present in every set."""
    from concourse.hw_specs import get_activation_tables

    tabs = get_activation_tables(arch)  # functools.cache -> shared dict
    keep = "natural_log_exp_and_others"
    if keep in tabs:
        for name, funcs in tabs.items():
            if name != keep:
                funcs.discard(mybir.ActivationFunctionType.Ln)
                funcs.discard(mybir.ActivationFunctionType.Exp)


def _build_body(tc, qT_d, kT_d, u_d, mask_d):
    nc = tc.nc
    n_rtiles = ROWS // P  # 8
    n_c = HD // P  # 4 contraction chunks
    act = mybir.ActivationFunctionType
    alu = mybir.AluOpType

    with (
        tc.tile_pool(name="consts", bufs=1) as consts,
        tc.tile_pool(name="kqT", bufs=1) as kqT_pool,
        tc.tile_pool(name="s_psum", bufs=2, space="PSUM") as s_psum,
        tc.tile_pool(name="g1p", bufs=2) as g1p,
        tc.tile_pool(name="g2p", bufs=5) as g2p,
        tc.tile_pool(name="zp", bufs=7) as zp,
        tc.tile_pool(name="uin", bufs=3) as uin,
        tc.tile_pool(name="mout", bufs=3) as mout,
        tc.tile_pool(name="small", bufs=7) as small,
    ):
        eps_tile = consts.tile([P, 1], F32)
        nc.vector.memset(eps_tile, EPS)

        u_t = u_d.rearrange("(t p) n -> t p n", p=P)
        mask_t = mask_d.rearrange("(t p) n -> t p n", p=P)

        # d-major fp16 loads straight from host-transposed DRAM; no PE transposes.
        kT_r = kT_d.rearrange("(c p) m -> c p m", p=P)
        qT_r = qT_d.rearrange("(c p) m -> c p m", p=P)
        kT = [kqT_pool.tile([P, N], F16, tag=f"kT{c}", name=f"kT{c}") for c in range(n_c)]
        qT = [kqT_pool.tile([P, ROWS], F16, tag=f"qT{c}", name=f"qT{c}") for c in range(n_c)]

        # Phase A: noise DMAs and the Ln stream, front-loaded so the in-order
        # ACT queue runs the 16 Ln passes back-to-back; tile 0's u load and
        # first-chunk weights go first so ACT/PE start as early as possible.
        ut_ = []
        for t in range(n_rtiles):
            ut = uin.tile([P, N], F32, tag="u")
            nc.sync.dma_start(out=ut, in_=u_t[t])
            ut_.append(ut)
            if t == 0:
                # weight chunks ride behind tile 0's noise; c-major so the
                # first matmuls can start after one chunk pair
                for c in range(n_c):
                    nc.sync.dma_start(out=kT[c], in_=kT_r[c])
                    nc.sync.dma_start(out=qT[c], in_=qT_r[c])

        g2_ = []
        for t in range(n_rtiles):
            g1 = g1p.tile([P, N], F32, tag="g1")
            nc.scalar.activation(g1, ut_[t], act.Ln, bias=eps_tile, scale=1.0)
            g2 = g2p.tile([P, N], F32, tag="g2")
            nc.scalar.activation(g2, g1, act.Ln, bias=eps_tile, scale=-1.0)
            g2_.append(g2)

        # Phase B: per tile: matmuls -> z combine -> segmented top-16
        # threshold; the last CMP_ON_DVE tiles' compares run on the DVE so
        # the kernel tail never waits on the ACT queue.
        CMP_ON_DVE = 2
        z_ = []
        sm_ = []
        for t in range(n_rtiles):
            M = s_psum.tile([P, N], F32, tag="M")  # 4 PSUM banks: 8*logits
            for c in range(n_c):
                for m in range(4):
                    nc.tensor.matmul(
                        M[:, m * 512 : (m + 1) * 512],
                        qT[c][:, t * P : (t + 1) * P],
                        kT[c][:, m * 512 : (m + 1) * 512],
                        start=(c == 0),
                        stop=(c == n_c - 1),
                    )

            # z = M/8 - g2: PSUM evac + head-mean + gumbel in one DVE op
            z = zp.tile([P, N], F32, tag="z")
            nc.vector.scalar_tensor_tensor(
                z, M, 0.125, g2_[t], alu.mult, alu.subtract
            )

            # top-8 of each 256-col segment -> 64 candidates; 16th largest of
            # candidates = row threshold. Merged small tile:
            # [0:64) cand | [64:72) c8a | [72:136) cand2 | [136:144) c8b
            # [144:145) tb = -t16m (Sign bias)
            sm = small.tile([P, 145], F32, tag="sm")
            for s in range(NSEG):
                nc.vector.max(
                    out=sm[:, 8 * s : 8 * (s + 1)], in_=z[:, SEG * s : SEG * (s + 1)]
                )
            nc.vector.max(out=sm[:, 64:72], in_=sm[:, 0:64])
            nc.vector.match_replace(
                out=sm[:, 72:136], in_to_replace=sm[:, 64:72],
                in_values=sm[:, 0:64], imm_value=NEG_BIG,
            )
            nc.vector.max(out=sm[:, 136:144], in_=sm[:, 72:136])
            nc.vector.tensor_scalar(
                out=sm[:, 144:145], in0=sm[:, 143:144],
                scalar1=T16_SHRINK, scalar2=None, op0=alu.mult,
            )
            z_.append(z)
            sm_.append(sm)

            if t >= n_rtiles - CMP_ON_DVE:
                mk = mout.tile([P, N], U8, tag="mk")
                nc.vector.tensor_scalar(
                    out=mk, in0=z, scalar1=sm[:, 143:144], scalar2=None,
                    op0=alu.is_ge,
                )
                nc.sync.dma_start(out=mask_t[t], in_=mk)

        # Phase C: masks for the remaining tiles on ACT (Sign clamps to {0,1}
        # on the u8 path); all dependencies are long since satisfied, so these
        # run back-to-back right after the Ln stream.
        for t in range(n_rtiles - CMP_ON_DVE):
            mk = mout.tile([P, N], U8, tag="mk")
            nc.scalar.activation(
                mk, z_[t], act.Sign, bias=sm_[t][:, 144:145], scale=1.0
            )
            nc.sync.dma_start(out=mask_t[t], in_=mk)


def build_kernel():
    nc = bacc.Bacc(
        "TRN2", target_bir_lowering=False, debug=False, num_devices=N_CORES
    )
    _pin_act_tables(nc.m.arch)
    qT = nc.dram_tensor("qT", [HD, ROWS], F16, kind="ExternalInput").ap()
    kT = nc.dram_tensor("kT", [HD, N], F16, kind="ExternalInput").ap()
    u = nc.dram_tensor("u", [ROWS, N], F32, kind="ExternalInput").ap()
    mask = nc.dram_tensor("mask", [ROWS, N], U8, kind="ExternalOutput").ap()
    with tile.TileContext(nc) as tc:
        _build_body(tc, qT, kT, u, mask)
    nc.compile()
    return nc


_NC_CACHE = None
LAST_RESULTS = None


def _get_nc():
    global _NC_CACHE
    if _NC_CACHE is None:
        _NC_CACHE = build_kernel()
    return _NC_CACHE


def make_in_maps(q, k, u):
    q = np.asarray(q, np.float32)
    k = np.asarray(k, np.float32)
    u = np.asarray(u, np.float32)
    in_maps = []
    kT_by_batch = {}
    for core in range(N_CORES):
        b, half = divmod(core, 2)
        r0 = half * ROWS
        if b not in kT_by_batch:
            # [N, H, D] -> [H*D, N] d-major, fp16
            kT_by_batch[b] = np.ascontiguousarray(
                k[b].transpose(1, 0, 2).reshape(N, HD).T
            ).astype(np.float16)
        # 1/8 = 1/sqrt(64) is an exact power of two: no extra rounding before
        # the fp16 cast; the head-mean 1/8 is applied in the z combine on-chip
        qT = np.ascontiguousarray(
            q[b, :, r0 : r0 + ROWS, :].transpose(1, 0, 2).reshape(ROWS, HD).T
            * np.float32(1.0 / 8)
        ).astype(np.float16)
        in_maps.append(
            {
                "qT": qT,
                "kT": kT_by_batch[b],
                "u": np.ascontiguousarray(u[b, r0 : r0 + ROWS]),
            }
        )
    return in_maps


def kernel(q, k, u):
    global LAST_RESULTS
    in_maps = make_in_maps(q, k, u)
    res = bass_utils.run_bass_kernel_spmd(
        _get_nc(), in_maps, core_ids=list(range(N_CORES))
    )
    LAST_RESULTS = res
    out = np.empty((B, 1, N, N), np.float32)
    for core in range(N_CORES):
        b, half = divmod(core, 2)
        r0 = half * ROWS
        out[b, 0, r0 : r0 + ROWS] = res.results[core]["mask"].astype(np.float32)
    return out


# revision 19
# speedup vs baseline: 1.0753x; 1.0753x over previous
"""Gumbel Top-K gate kernel for Trainium2 (8 NeuronCores, SPMD).

Math: mask[b, 0, r, m] = 1 iff z[b, r, m] is among the top-16 of row r, where
  z = mean_h(q_h k_h^T)/sqrt(64) + gumbel(u),  gumbel = -log(-log(u+eps)+eps).
Softmax is strictly monotone per row, so the reference's softmax/top-k mask
equals thresholding z at its 16th-largest value per row (ties included via >=).

Engine split (the DVE and GpSimd share an SBUF port, so GpSimd offload is
counterproductive — GpSimd is unused; ACT has its own ports):
  ACT: g1 = Ln(u+eps); g2 = Ln(-g1+eps); final compare as
       mask_u8 = Sign(z - t16m) (Sign clamps {-1,0,1} -> {0,1} on the u8
       output path; t16m is 1-2 ulp below t16 so z == t16 lands at 1).
  PE:  16 fp16 matmuls per 128-row tile (q pre-scaled by 1/8 on host; PSUM
       holds 8*logits).
  DVE: one fused scalar_tensor_tensor z = (PSUM*0.125) - g2 (PSUM
       evacuation + head-mean + gumbel combine in one 1x pass), then the
       top-k search: 8x max8 over 256-col segments -> 64 candidates, then
       max8/match_replace/max8 on the candidates -> t16 = 16th largest.

Segmented max8 is exact unless one segment holds >8 of the row's top-16
(P ~ 4e-5 per row). Measured flips vs the fp32 reference on the actual
inputs: 7 of 16.7M (rel err 7.3e-3, budget 2e-2) — from the fp16 matmul
(6) and the segment assumption (1).

The mask stage for tile t is emitted after tile t+1's Ln passes (manual
software pipelining) so the in-order ACT queue never stalls waiting for the
DVE chain. Activation tables are pinned to one set to avoid reload thrash.

Sharding: core c handles batch b = c//2, row half c%2 -> [1024, 2048] slab.
"""

import sys

sys.path.insert(0, "/opt/trn_rl_repo")

import numpy as np

import concourse.bacc as bacc
import concourse.mybir as mybir
import concourse.tile as tile
from concourse import bass_utils

B, H, N, D = 4, 8, 2048, 64
HD = H * D  # 512 contraction dim (heads concatenated)
N_CORES = 8
ROWS = N * B // N_CORES  # 1024 rows per core
P = 128
EPS = 1e-9
NEG_BIG = -3.0e38
NSEG = 8
SEG = N // NSEG  # 256
# t16m = t16 * (1 - 2^-22): 1-2 ulp below t16 (t16 > 0 w.o.p.), so
# Sign(z - t16m) is +1 exactly when z >= t16.
T16_SHRINK = -(1.0 - 2.0**-22)  # negated: used directly as the Sign bias
F32 = mybir.dt.float32
F16 = mybir.dt.float16
U8 = mybir.dt.uint8


def _pin_act_tables(arch):
    """Pin Ln (and Exp, if ever used) to the combined table set so the ACT
    engine never reloads tables mid-kernel; Sign is present in every set."""
    from concourse.hw_specs import get_activation_tables

    tabs = get_activation_tables(arch)  # functools.cache -> shared dict
    keep = "natural_log_exp_and_others"
    if keep in tabs:
        for name, funcs in tabs.items():
            if name != keep:
                funcs.discard(mybir.ActivationFunctionType.Ln)
                funcs.discard(mybir.ActivationFunctionType.Exp)


def _build_body(tc, qT_d, kT_d, u_d, mask_d):
    nc = tc.nc
    n_rtiles = ROWS // P  # 8
    n_c = HD // P  # 4 contraction chunks
    act = mybir.ActivationFunctionType
    alu = mybir.AluOpType

    CMP_ON_DVE = 2  # last tiles' compares on the DVE: no tail wait on ACT

    with (
        tc.tile_pool(name="consts", bufs=1) as consts,
        tc.tile_pool(name="kqT", bufs=1) as kqT_pool,
        tc.tile_pool(name="s_psum", bufs=2, space="PSUM") as s_psum,
        tc.tile_pool(name="work", bufs=3) as work,
        tc.tile_pool(name="uin", bufs=3) as uin,
        tc.tile_pool(name="mout", bufs=2) as mout,
        tc.tile_pool(name="small", bufs=2) as small,
    ):
        eps_tile = consts.tile([P, 1], F32)
        nc.vector.memset(eps_tile, EPS)

        u_t = u_d.rearrange("(t p) n -> t p n", p=P)
        mask_t = mask_d.rearrange("(t p) n -> t p n", p=P)

        # d-major fp16 loads straight from host-transposed DRAM; no PE transposes.
        kT_r = kT_d.rearrange("(c p) m -> c p m", p=P)
        qT_r = qT_d.rearrange("(c p) m -> c p m", p=P)
        kT = [kqT_pool.tile([P, N], F16, tag=f"kT{c}", name=f"kT{c}") for c in range(n_c)]
        qT = [kqT_pool.tile([P, ROWS], F16, tag=f"qT{c}", name=f"qT{c}") for c in range(n_c)]

        # Input DMA order matters: tile 0's noise goes first (in two halves so
        # the first Ln can start sooner), then weight chunks interleaved with
        # the next noise tiles so neither the ACT stream nor the PE starves.
        ut_ = {}
        ut_[0] = uin.tile([P, N], F32, tag="u", name="ut0")
        nc.sync.dma_start(out=ut_[0][:, : N // 2], in_=u_t[0][:, : N // 2])
        nc.sync.dma_start(out=ut_[0][:, N // 2 :], in_=u_t[0][:, N // 2 :])
        for c in range(n_c):
            nc.sync.dma_start(out=kT[c], in_=kT_r[c])
            nc.sync.dma_start(out=qT[c], in_=qT_r[c])
            if c + 1 < n_rtiles and c + 1 <= 2:
                ut_[c + 1] = uin.tile([P, N], F32, tag="u", name=f"ut{c+1}")
                nc.sync.dma_start(out=ut_[c + 1], in_=u_t[c + 1])

        pending = None  # previous tile's mask stage, emitted late so the
        # in-order ACT queue never stalls on the DVE chain

        def emit_mask(zp_, smp, tp):
            mk = mout.tile([P, N], U8, tag="mk")
            nc.scalar.activation(mk, zp_, act.Sign, bias=smp[:, 144:145], scale=1.0)
            nc.sync.dma_start(out=mask_t[tp], in_=mk)

        for t in range(n_rtiles):
            if t not in ut_:
                ut_[t] = uin.tile([P, N], F32, tag="u", name=f"ut{t}")
                nc.sync.dma_start(out=ut_[t], in_=u_t[t])
            ut = ut_[t]
            g1 = work.tile([P, N], F32, tag="g1")
            g2 = work.tile([P, N], F32, tag="g2")
            if t == 0:
                # two half-width Ln passes so ACT starts on the first half DMA
                for hh in range(2):
                    cols = slice(hh * (N // 2), (hh + 1) * (N // 2))
                    nc.scalar.activation(g1[:, cols], ut[:, cols], act.Ln, bias=eps_tile, scale=1.0)
                    nc.scalar.activation(g2[:, cols], g1[:, cols], act.Ln, bias=eps_tile, scale=-1.0)
            else:
                nc.scalar.activation(g1, ut, act.Ln, bias=eps_tile, scale=1.0)
                nc.scalar.activation(g2, g1, act.Ln, bias=eps_tile, scale=-1.0)

            M = s_psum.tile([P, N], F32, tag="M")  # 4 PSUM banks, holds 8*logits
            for c in range(n_c):
                for m in range(4):
                    nc.tensor.matmul(
                        M[:, m * 512 : (m + 1) * 512],
                        qT[c][:, t * P : (t + 1) * P],
                        kT[c][:, m * 512 : (m + 1) * 512],
                        start=(c == 0),
                        stop=(c == n_c - 1),
                    )

            # z = M/8 - g2: PSUM evacuation + head-mean + gumbel in one DVE op
            z = work.tile([P, N], F32, tag="z")
            nc.vector.scalar_tensor_tensor(
                z, M, 0.125, g2, alu.mult, alu.subtract
            )

            # top-8 of each 256-col segment -> 64 candidates; 16th largest of
            # candidates = row threshold. Merged small tile:
            # [0:64) cand | [64:72) c8a | [72:136) cand2 | [136:144) c8b
            # [144:145) tb = -t16m (Sign bias)
            sm = small.tile([P, 145], F32, tag="sm")
            for s in range(NSEG):
                nc.vector.max(out=sm[:, 8 * s : 8 * (s + 1)], in_=z[:, SEG * s : SEG * (s + 1)])
            nc.vector.max(out=sm[:, 64:72], in_=sm[:, 0:64])
            nc.vector.match_replace(
                out=sm[:, 72:136], in_to_replace=sm[:, 64:72],
                in_values=sm[:, 0:64], imm_value=NEG_BIG,
            )
            nc.vector.max(out=sm[:, 136:144], in_=sm[:, 72:136])
            nc.vector.tensor_scalar(
                out=sm[:, 144:145], in0=sm[:, 143:144],
                scalar1=T16_SHRINK, scalar2=None, op0=alu.mult,
            )

            if t >= n_rtiles - CMP_ON_DVE:
                # tail tiles: compare on the DVE right after the chain
                mk = mout.tile([P, N], U8, tag="mk")
                nc.vector.tensor_scalar(
                    out=mk, in0=z, scalar1=sm[:, 143:144], scalar2=None,
                    op0=alu.is_ge,
                )
                nc.sync.dma_start(out=mask_t[t], in_=mk)
            else:
                if pending is not None:
                    emit_mask(*pending)
                pending = (z, sm, t)

        if pending is not None:
            emit_mask(*pending)


def build_kernel():
    nc = bacc.Bacc(
        "TRN2", target_bir_lowering=False, debug=False, num_devices=N_CORES
    )
    _pin_act_tables(nc.m.arch)
    qT = nc.dram_tensor("qT", [HD, ROWS], F16, kind="ExternalInput").ap()
    kT = nc.dram_tensor("kT", [HD, N], F16, kind="ExternalInput").ap()
    u = nc.dram_tensor("u", [ROWS, N], F32, kind="ExternalInput").ap()
    mask = nc.dram_tensor("mask", [ROWS, N], U8, kind="ExternalOutput").ap()
    with tile.TileContext(nc) as tc:
        _build_body(tc, qT, kT, u, mask)
    nc.compile()
    return nc


_NC_CACHE = None
LAST_RESULTS = None


def _get_nc():
    global _NC_CACHE
    if _NC_CACHE is None:
        _NC_CACHE = build_kernel()
    return _NC_CACHE


def make_in_maps(q, k, u):
    q = np.asarray(q, np.float32)
    k = np.asarray(k, np.float32)
    u = np.asarray(u, np.float32)
    in_maps = []
    kT_by_batch = {}
    for core in range(N_CORES):
        b, half = divmod(core, 2)
        r0 = half * ROWS
        if b not in kT_by_batch:
            # [N, H, D] -> [H*D, N] d-major, fp16
            kT_by_batch[b] = np.ascontiguousarray(
                k[b].transpose(1, 0, 2).reshape(N, HD).T
            ).astype(np.float16)
        # 1/8 = 1/sqrt(64) is an exact power of two: no extra rounding before
        # the fp16 cast; the head-mean 1/8 is applied in the z combine on-chip
        qT = np.ascontiguousarray(
            q[b, :, r0 : r0 + ROWS, :].transpose(1, 0, 2).reshape(ROWS, HD).T
            * np.float32(1.0 / 8)
        ).astype(np.float16)
        in_maps.append(
            {
                "qT": qT,
                "kT": kT_by_batch[b],
                "u": np.ascontiguousarray(u[b, r0 : r0 + ROWS]),
            }
        )
    return in_maps


def kernel(q, k, u):
    global LAST_RESULTS
    in_maps = make_in_maps(q, k, u)
    res = bass_utils.run_bass_kernel_spmd(
        _get_nc(), in_maps, core_ids=list(range(N_CORES))
    )
    LAST_RESULTS = res
    out = np.empty((B, 1, N, N), np.float32)
    for core in range(N_CORES):
        b, half = divmod(core, 2)
        r0 = half * ROWS
        out[b, 0, r0 : r0 + ROWS] = res.results[core]["mask"].astype(np.float32)
    return out


# revision 20
# speedup vs baseline: 1.0826x; 1.0068x over previous
"""Gumbel Top-K gate kernel for Trainium2 (8 NeuronCores, SPMD).

Math: mask[b, 0, r, m] = 1 iff z[b, r, m] is among the top-16 of row r, where
  z = mean_h(q_h k_h^T)/sqrt(64) + gumbel(u),  gumbel = -log(-log(u+eps)+eps).
Softmax is strictly monotone per row, so the reference's softmax/top-k mask
equals thresholding z at its 16th-largest value per row (ties included via >=).

Engine split (the DVE and GpSimd share an SBUF port, so GpSimd offload is
counterproductive — GpSimd is unused; ACT has its own ports):
  ACT: g1 = Ln(u+eps); g2 = Ln(-g1+eps); final compare as
       mask_u8 = Sign(z - t16m) (Sign clamps {-1,0,1} -> {0,1} on the u8
       output path; t16m is 1-2 ulp below t16 so z == t16 lands at 1).
  PE:  16 fp16 matmuls per 128-row tile (q pre-scaled by 1/8 on host; PSUM
       holds 8*logits).
  DVE: one fused scalar_tensor_tensor z = (PSUM*0.125) - g2 (PSUM
       evacuation + head-mean + gumbel combine in one 1x pass), then the
       top-k search: 8x max8 over 256-col segments -> 64 candidates, then
       max8/match_replace/max8 on the candidates -> t16 = 16th largest.

Segmented max8 is exact unless one segment holds >8 of the row's top-16
(P ~ 4e-5 per row). Measured flips vs the fp32 reference on the actual
inputs: 7 of 16.7M (rel err 7.3e-3, budget 2e-2) — from the fp16 matmul
(6) and the segment assumption (1).

The mask stage for tile t is emitted after tile t+1's Ln passes (manual
software pipelining) so the in-order ACT queue never stalls waiting for the
DVE chain. Activation tables are pinned to one set to avoid reload thrash.

Sharding: core c handles batch b = c//2, row half c%2 -> [1024, 2048] slab.
"""

import sys

sys.path.insert(0, "/opt/trn_rl_repo")

import numpy as np

import concourse.bacc as bacc
import concourse.mybir as mybir
import concourse.tile as tile
from concourse import bass_utils

B, H, N, D = 4, 8, 2048, 64
HD = H * D  # 512 contraction dim (heads concatenated)
N_CORES = 8
ROWS = N * B // N_CORES  # 1024 rows per core
P = 128
EPS = 1e-9
NEG_BIG = -3.0e38
NSEG = 8
SEG = N // NSEG  # 256
# t16m = t16 * (1 - 2^-22): 1-2 ulp below t16 (t16 > 0 w.o.p.), so
# Sign(z - t16m) is +1 exactly when z >= t16.
T16_SHRINK = -(1.0 - 2.0**-22)  # negated: used directly as the Sign bias
F32 = mybir.dt.float32
F16 = mybir.dt.float16
U8 = mybir.dt.uint8


def _pin_act_tables(arch):
    """Pin Ln (and Exp, if ever used) to the combined table set so the ACT
    engine never reloads tables mid-kernel; Sign is present in every set."""
    from concourse.hw_specs import get_activation_tables

    tabs = get_activation_tables(arch)  # functools.cache -> shared dict
    keep = "natural_log_exp_and_others"
    if keep in tabs:
        for name, funcs in tabs.items():
            if name != keep:
                funcs.discard(mybir.ActivationFunctionType.Ln)
                funcs.discard(mybir.ActivationFunctionType.Exp)


def _build_body(tc, qT_d, kT_d, u_d, mask_d):
    nc = tc.nc
    n_rtiles = ROWS // P  # 8
    n_c = HD // P  # 4 contraction chunks
    act = mybir.ActivationFunctionType
    alu = mybir.AluOpType

    CMP_ON_DVE = 2  # last tiles' compares on the DVE: no tail wait on ACT

    with (
        tc.tile_pool(name="consts", bufs=1) as consts,
        tc.tile_pool(name="kqT", bufs=1) as kqT_pool,
        tc.tile_pool(name="s_psum", bufs=2, space="PSUM") as s_psum,
        tc.tile_pool(name="work", bufs=3) as work,
        tc.tile_pool(name="uin", bufs=3) as uin,
        tc.tile_pool(name="mout", bufs=2) as mout,
        tc.tile_pool(name="small", bufs=2) as small,
    ):
        eps_tile = consts.tile([P, 1], F32)
        nc.vector.memset(eps_tile, EPS)

        u_t = u_d.rearrange("(t p) n -> t p n", p=P)
        mask_t = mask_d.rearrange("(t p) n -> t p n", p=P)

        # d-major fp16 loads straight from host-transposed DRAM; no PE transposes.
        kT_r = kT_d.rearrange("(c p) m -> c p m", p=P)
        qT_r = qT_d.rearrange("(c p) m -> c p m", p=P)
        kT = [kqT_pool.tile([P, N], F16, tag=f"kT{c}", name=f"kT{c}") for c in range(n_c)]
        qT = [kqT_pool.tile([P, ROWS], F16, tag=f"qT{c}", name=f"qT{c}") for c in range(n_c)]

        # Input DMA order: ALL weight chunks first — M(0) needs every
        # contraction chunk, and from tile ~4 on the whole pipeline is paced
        # by the (power-throttled) PE, so the PE must start as early as
        # possible. The u path (DMA + 2 Ln) catches up by tile 3-4.
        for c in range(n_c):
            nc.sync.dma_start(out=kT[c], in_=kT_r[c])
            nc.sync.dma_start(out=qT[c], in_=qT_r[c])
        ut_ = {}
        ut_[0] = uin.tile([P, N], F32, tag="u", name="ut0")
        nc.sync.dma_start(out=ut_[0][:, : N // 2], in_=u_t[0][:, : N // 2])
        nc.sync.dma_start(out=ut_[0][:, N // 2 :], in_=u_t[0][:, N // 2 :])
        for t in (1, 2):
            ut_[t] = uin.tile([P, N], F32, tag="u", name=f"ut{t}")
            nc.sync.dma_start(out=ut_[t], in_=u_t[t])

        pending = None  # previous tile's mask stage, emitted late so the
        # in-order ACT queue never stalls on the DVE chain

        def emit_mask(zp_, smp, tp):
            mk = mout.tile([P, N], U8, tag="mk")
            nc.scalar.activation(mk, zp_, act.Sign, bias=smp[:, 144:145], scale=1.0)
            nc.sync.dma_start(out=mask_t[tp], in_=mk)

        for t in range(n_rtiles):
            if t not in ut_:
                ut_[t] = uin.tile([P, N], F32, tag="u", name=f"ut{t}")
                nc.sync.dma_start(out=ut_[t], in_=u_t[t])
            ut = ut_[t]
            g1 = work.tile([P, N], F32, tag="g1")
            g2 = work.tile([P, N], F32, tag="g2")
            if t == 0:
                # two half-width Ln passes so ACT starts on the first half DMA
                for hh in range(2):
                    cols = slice(hh * (N // 2), (hh + 1) * (N // 2))
                    nc.scalar.activation(g1[:, cols], ut[:, cols], act.Ln, bias=eps_tile, scale=1.0)
                    nc.scalar.activation(g2[:, cols], g1[:, cols], act.Ln, bias=eps_tile, scale=-1.0)
            else:
                nc.scalar.activation(g1, ut, act.Ln, bias=eps_tile, scale=1.0)
                nc.scalar.activation(g2, g1, act.Ln, bias=eps_tile, scale=-1.0)

            M = s_psum.tile([P, N], F32, tag="M")  # 4 PSUM banks, holds 8*logits
            for c in range(n_c):
                for m in range(4):
                    nc.tensor.matmul(
                        M[:, m * 512 : (m + 1) * 512],
                        qT[c][:, t * P : (t + 1) * P],
                        kT[c][:, m * 512 : (m + 1) * 512],
                        start=(c == 0),
                        stop=(c == n_c - 1),
                    )

            # z = M/8 - g2: PSUM evacuation + head-mean + gumbel in one DVE op
            z = work.tile([P, N], F32, tag="z")
            nc.vector.scalar_tensor_tensor(
                z, M, 0.125, g2, alu.mult, alu.subtract
            )

            # top-8 of each 256-col segment -> 64 candidates; 16th largest of
            # candidates = row threshold. Merged small tile:
            # [0:64) cand | [64:72) c8a | [72:136) cand2 | [136:144) c8b
            # [144:145) tb = -t16m (Sign bias)
            sm = small.tile([P, 145], F32, tag="sm")
            for s in range(NSEG):
                nc.vector.max(out=sm[:, 8 * s : 8 * (s + 1)], in_=z[:, SEG * s : SEG * (s + 1)])
            nc.vector.max(out=sm[:, 64:72], in_=sm[:, 0:64])
            nc.vector.match_replace(
                out=sm[:, 72:136], in_to_replace=sm[:, 64:72],
                in_values=sm[:, 0:64], imm_value=NEG_BIG,
            )
            nc.vector.max(out=sm[:, 136:144], in_=sm[:, 72:136])
            nc.vector.tensor_scalar(
                out=sm[:, 144:145], in0=sm[:, 143:144],
                scalar1=T16_SHRINK, scalar2=None, op0=alu.mult,
            )

            if t >= n_rtiles - CMP_ON_DVE:
                # tail tiles: compare on the DVE right after the chain
                mk = mout.tile([P, N], U8, tag="mk")
                nc.vector.tensor_scalar(
                    out=mk, in0=z, scalar1=sm[:, 143:144], scalar2=None,
                    op0=alu.is_ge,
                )
                nc.sync.dma_start(out=mask_t[t], in_=mk)
            else:
                if pending is not None:
                    emit_mask(*pending)
                pending = (z, sm, t)

        if pending is not None:
            emit_mask(*pending)


def build_kernel():
    nc = bacc.Bacc(
        "TRN2", target_bir_lowering=False, debug=False, num_devices=N_CORES
    )
    _pin_act_tables(nc.m.arch)
    qT = nc.dram_tensor("qT", [HD, ROWS], F16, kind="ExternalInput").ap()
    kT = nc.dram_tensor("kT", [HD, N], F16, kind="ExternalInput").ap()
    u = nc.dram_tensor("u", [ROWS, N], F32, kind="ExternalInput").ap()
    mask = nc.dram_tensor("mask", [ROWS, N], U8, kind="ExternalOutput").ap()
    with tile.TileContext(nc) as tc:
        _build_body(tc, qT, kT, u, mask)
    nc.compile()
    return nc


_NC_CACHE = None
LAST_RESULTS = None


def _get_nc():
    global _NC_CACHE
    if _NC_CACHE is None:
        _NC_CACHE = build_kernel()
    return _NC_CACHE


def make_in_maps(q, k, u):
    q = np.asarray(q, np.float32)
    k = np.asarray(k, np.float32)
    u = np.asarray(u, np.float32)
    in_maps = []
    kT_by_batch = {}
    for core in range(N_CORES):
        b, half = divmod(core, 2)
        r0 = half * ROWS
        if b not in kT_by_batch:
            # [N, H, D] -> [H*D, N] d-major, fp16
            kT_by_batch[b] = np.ascontiguousarray(
                k[b].transpose(1, 0, 2).reshape(N, HD).T
            ).astype(np.float16)
        # 1/8 = 1/sqrt(64) is an exact power of two: no extra rounding before
        # the fp16 cast; the head-mean 1/8 is applied in the z combine on-chip
        qT = np.ascontiguousarray(
            q[b, :, r0 : r0 + ROWS, :].transpose(1, 0, 2).reshape(ROWS, HD).T
            * np.float32(1.0 / 8)
        ).astype(np.float16)
        in_maps.append(
            {
                "qT": qT,
                "kT": kT_by_batch[b],
                "u": np.ascontiguousarray(u[b, r0 : r0 + ROWS]),
            }
        )
    return in_maps


def kernel(q, k, u):
    global LAST_RESULTS
    in_maps = make_in_maps(q, k, u)
    res = bass_utils.run_bass_kernel_spmd(
        _get_nc(), in_maps, core_ids=list(range(N_CORES))
    )
    LAST_RESULTS = res
    out = np.empty((B, 1, N, N), np.float32)
    for core in range(N_CORES):
        b, half = divmod(core, 2)
        r0 = half * ROWS
        out[b, 0, r0 : r0 + ROWS] = res.results[core]["mask"].astype(np.float32)
    return out
